# revision 19
# baseline (speedup 1.0000x reference)
"""Trainium2 Bass kernel for nn_Attention (dense transformer spatial attention).

Reference computation (per batch b):
    q = Wq @ x   (1x1 conv over channels), k = Wk @ c, v = Wv @ c
    per head h (8 heads, head_dim 32, n = 64*64 = 4096 tokens):
        S = (q_h^T k_h) * DIM**-0.5 ; P = softmax(S, axis=-1) ; o_h = v_h P^T
    out = Wo @ concat(o_h)

Sharding (8 cores): core c handles batch b = c//2 and heads 4*(c%2) .. +4
(tensor-parallel over heads).  The two cores of a batch produce partial
outputs Y = Wo_slice @ o_slice which the host sums (fp16 partials, fp32 sum).

Per-core dataflow:
  - The exp() of softmax is the bottleneck: 67M elements/core must be read
    from fp32 PSUM by ScalarE/VectorE (the only engines with PSUM access).
    Work is split: ScalarE runs native exp ACTIVATE on heads {0,1};
    VectorE computes heads {2,3} via a one-instruction Schraudolph exp2:
    the scores arrive pre-scaled by 1024*log2(e)*DIM^-0.5 (folded into Wq
    host-side), so  int16(z + B)  reinterpreted as fp16 bits IS exp(s).
    Every SPLIT_K-th round ScalarE also takes heads {2,3} to balance load.
  - Scores are computed transposed S^T[j,i] with 4-way row-tiled matmuls
    (K=32 per head, tile_position=(32h,0)), 4 slots -> 4 distinct PSUM banks.
  - vT (token-major V with a ones column for the softmax denominator) is
    produced directly by a transposed projection: lhsT=context chunk,
    rhs=Wv^T chunk -> out[token, (h,d)].  No PE transposes needed.
  - AV: col-tiled matmuls (M=33: 32 dims + ones row) accumulate over j in
    fp32 PSUM; normalization by 1/l on evacuation (DVE recip + gpsimd
    broadcast + DVE mul).
  - Output projection: fp16 matmul with WoT; fp16 partials DMA'd out.
"""

import os
import sys

import numpy as np

for _p in ("/opt/trn_rl_repo", "/root/.axon_site/_ro/trn_rl_repo"):
    if os.path.isdir(_p) and _p not in sys.path:
        sys.path.insert(0, _p)

import concourse.bass as bass
import concourse.tile as tile
from concourse import bacc, mybir
from concourse.bass import ts
from concourse.bass_utils import run_bass_kernel_spmd

DIM = 512
HEAD = 8
ATTN_DIM = 256
HEAD_DIM = 32
N = 4096  # 64 * 64 tokens
SCALE = DIM ** -0.5

N_CORES = 8
HEADS_PER_CORE = 4
NI = 512   # i-tile (query tokens per score matmul rhs)
NJ = 128   # j-tile (key tokens per score matmul lhsT)
AV_LAG = 2  # rounds between score emission and its AV consumption

# exp2 prescale folded into Wq: scores arrive as z = 1024*log2(e)*s
PRESCALE = 1024.0 * np.log2(np.e) * SCALE
EXP_SCALE = float(np.log(2.0) / 1024.0)   # ScalarE: exp(z * EXP_SCALE) = exp(s)
SCH_B = 15315.0                            # DVE: fp16bits(exp(s)) ~= int16(z + B)
SPLIT_K = 10                               # every k-th round ScalarE takes tB too

F32 = mybir.dt.float32
F16 = mybir.dt.float16
I16 = mybir.dt.int16


def build_nc():
    nc = bacc.Bacc()

    x_d = nc.dram_tensor("x", [DIM, N], F16, kind="ExternalInput").ap()
    c_d = nc.dram_tensor("c", [DIM, N], F16, kind="ExternalInput").ap()
    wqt_d = nc.dram_tensor("wqt", [DIM, 128], F16, kind="ExternalInput").ap()
    wkt_d = nc.dram_tensor("wkt", [DIM, 128], F16, kind="ExternalInput").ap()
    wvt_d = nc.dram_tensor("wvt", [DIM, 128], F16, kind="ExternalInput").ap()
    wot_d = nc.dram_tensor("wot", [128, DIM], F16, kind="ExternalInput").ap()
    y_d = nc.dram_tensor("y", [DIM, N], F16, kind="ExternalOutput").ap()

    from contextlib import ExitStack

    with tile.TileContext(nc) as tc, ExitStack() as stk:
        persist = stk.enter_context(tc.tile_pool(name="persist", bufs=1))

        q_sb = persist.tile([128, N], F16)
        k_sb = persist.tile([128, N], F16)
        # vT: (token-in-chunk, j_chunk, head, 32 dims + ones col)
        vT_sb = persist.tile([128, N // NJ, HEADS_PER_CORE, HEAD_DIM + 1], F16)
        wot_sb = persist.tile([128, DIM], F16)

        nc.sync.dma_start(out=wot_sb, in_=wot_d)
        nc.vector.memset(vT_sb[:, :, :, HEAD_DIM:], 1.0)

        # Preload the exp activation table during the DMA lead-in so the
        # first real exp doesn't pay the ~2.7us ACT_TABLE_LOAD.
        warm_sb = persist.tile([1, 32], F32)
        nc.vector.memset(warm_sb, 0.0)
        nc.scalar.activation(out=warm_sb, in_=warm_sb,
                             func=mybir.ActivationFunctionType.Exp)

        # ---------------- Phase 1: projections ----------------
        cw_pool = stk.enter_context(tc.tile_pool(name="cw", bufs=1))

        w_sb = {}
        for nm, d in (("wkt", wkt_d), ("wqt", wqt_d), ("wvt", wvt_d)):
            w = cw_pool.tile([128, 4, 128], F16, tag=nm)
            nc.sync.dma_start(out=w, in_=d.rearrange("(c p) m -> p c m", p=128))
            w_sb[nm] = w
        # Inputs arrive as [128, 1024] piece tiles spread over four engine
        # DMA queues so projections start within ~2us instead of waiting for
        # full 1MB chunks on one queue.
        n_w = N // (2 * NI)  # 4 pieces per channel-chunk
        c_t = [[None] * n_w for _ in range(4)]
        x_t = [[None] * n_w for _ in range(4)]

        with tc.tile_pool(name="x_in", bufs=1) as x_pool, \
             tc.tile_pool(name="pj_ps", bufs=2, space="PSUM") as pj_ps, \
             tc.tile_pool(name="vt_ps", bufs=4, space="PSUM") as vt_ps:
            for w in range(n_w):
                for cc in range(4):
                    t = cw_pool.tile([128, 2 * NI], F16, tag="c_in", bufs=16)
                    q_eng = nc.sync if cc % 2 == 0 else nc.scalar
                    q_eng.dma_start(out=t, in_=c_d[ts(cc, 128), ts(w, 2 * NI)])
                    c_t[cc][w] = t
                    t = x_pool.tile([128, 2 * NI], F16, tag="x_in", bufs=16)
                    nc.gpsimd.dma_start(out=t, in_=x_d[ts(cc, 128), ts(w, 2 * NI)])
                    x_t[cc][w] = t

            def project_tile(wname, src, dst, t):
                w = w_sb[wname]
                ps = pj_ps.tile([128, NI], F32, tag="pj")
                for cc in range(4):
                    nc.tensor.matmul(
                        ps, lhsT=w[:, cc, :],
                        rhs=src[cc][t // 2][:, ts(t % 2, NI)],
                        start=(cc == 0), stop=(cc == 3),
                    )
                # evacuation split across both PSUM-capable engines
                if t % 2 == 0:
                    nc.scalar.copy(out=dst[:, ts(t, NI)], in_=ps)
                else:
                    nc.vector.tensor_copy(out=dst[:, ts(t, NI)], in_=ps)

            for t in range(N // NI):
                project_tile("wkt", c_t, k_sb, t)
                project_tile("wqt", x_t, q_sb, t)

            # vT via transposed projection: out[token, (h,d)]
            wv = w_sb["wvt"]
            for t in range(N // NJ):
                ps = vt_ps.tile([128, 128], F32, tag="vt")
                for cc in range(4):
                    nc.tensor.matmul(
                        ps,
                        lhsT=c_t[cc][t // 8][:, ts(t % 8, NJ)],
                        rhs=wv[:, cc, :],
                        start=(cc == 0), stop=(cc == 3),
                    )
                src = ps.rearrange("p (h d) -> p h d", h=HEADS_PER_CORE)
                if t % 2 == 0:
                    nc.scalar.copy(out=vT_sb[:, t, :, 0:HEAD_DIM], in_=src)
                else:
                    nc.vector.tensor_copy(out=vT_sb[:, t, :, 0:HEAD_DIM], in_=src)

        # ---------------- Phase 2: attention ----------------
        with tc.tile_pool(name="sc_ps", bufs=3, space="PSUM") as sc_ps, \
             tc.tile_pool(name="av_ps", bufs=2, space="PSUM") as av_ps, \
             tc.tile_pool(name="attn", bufs=1) as at_pool:

            n_i = N // NI   # 8
            n_j = N // NJ   # 32

            pending_out = []
            round_no = [0]

            def flush_outproj():
                while pending_out:
                    oi, rsb = pending_out.pop(0)
                    for half in range(2):
                        ps = sc_ps.tile([128, 2 * NI], F32, tag="sc",
                                        name=f"ofps_{oi}_{half}")
                        for cc in range(2):
                            c4 = 2 * half + cc
                            nc.tensor.matmul(
                                ps[:, ts(cc, NI)],
                                lhsT=wot_sb[:, ts(c4, 128)], rhs=rsb,
                                start=True, stop=True,
                            )
                        ot = at_pool.tile([128, 2 * NI], F16, tag="ot", bufs=4,
                                          name=f"ot_{oi}_{half}")
                        if half == 0:
                            nc.scalar.copy(out=ot, in_=ps)
                        else:
                            nc.vector.tensor_copy(out=ot, in_=ps)
                        for cc in range(2):
                            c4 = 2 * half + cc
                            nc.sync.dma_start(
                                out=y_d[ts(c4, 128), ts(oi, NI)],
                                in_=ot[:, ts(cc, NI)])

            pending_norm = [None]

            for i in range(n_i):
                raw_sb = at_pool.tile([128, NI], F16, tag="raw", bufs=2)
                es_ref = [None] * n_j  # j -> (esA, esB)
                acc = [None, None]

                def emit_round(j):
                    # alternate (tA, tB) allocation order so the pool
                    # rotation's 3-back dependency lands on the same engine
                    # half the time instead of always cross-coupling S and D.
                    tA = sc_ps.tile([128, 2 * NI], F32, tag="sc",
                                    name=f"scA_{i}_{j}")
                    tB = sc_ps.tile([128, 2 * NI], F32, tag="sc",
                                    name=f"scB_{i}_{j}")
                    if j % 2 == 1:
                        tA, tB = tB, tA
                    # emit the cross-rotation-blocked tile's matmuls first so
                    # its exp starts as early as possible (even j: tA waits on
                    # the other engine; odd j: tB does)
                    h_order = (0, 1, 2, 3) if j % 2 == 0 else (2, 3, 0, 1)
                    for h in h_order:
                        dst = (tA if h < 2 else tB)[:, ts(h % 2, NI)]
                        base = 32 * h
                        nc.tensor.matmul(
                            dst,
                            lhsT=k_sb[base:base + 32, ts(j, NJ)],
                            rhs=q_sb[base:base + 32, ts(i, NI)],
                            start=True, stop=True,
                            tile_position=(base, 0),
                        )
                    esA = at_pool.tile([128, 2 * NI], F16, tag="es",
                                       bufs=12, name=f"esA_{i}_{j}")
                    esB = at_pool.tile([128, 2 * NI], F16, tag="es",
                                       bufs=12, name=f"esB_{i}_{j}")
                    nc.scalar.activation(
                        out=esA, in_=tA,
                        func=mybir.ActivationFunctionType.Exp,
                        scale=EXP_SCALE,
                    )
                    r = round_no[0]
                    round_no[0] += 1
                    if r % SPLIT_K == SPLIT_K - 1:
                        nc.scalar.activation(
                            out=esB, in_=tB,
                            func=mybir.ActivationFunctionType.Exp,
                            scale=EXP_SCALE,
                        )
                    else:
                        nc.vector.tensor_scalar(
                            out=esB.bitcast(I16), in0=tB,
                            scalar1=SCH_B, scalar2=None,
                            op0=mybir.AluOpType.add,
                        )
                    es_ref[j] = (esA, esB)

                def emit_av(j):
                    esA, esB = es_ref[j]
                    for p in range(2):
                        es = esA if p == 0 else esB
                        for hh in range(2):
                            nc.tensor.matmul(
                                acc[p][64 * hh:64 * hh + HEAD_DIM + 1, :],
                                lhsT=vT_sb[:, j, 2 * p + hh, :],
                                rhs=es[:, ts(hh, NI)],
                                start=(j == 0), stop=(j == n_j - 1),
                                tile_position=(0, 64 * hh),
                                skip_group_check=True,
                            )

                for j in range(n_j):
                    emit_round(j)
                    if j == 1 and pending_norm[0] is not None:
                        pending_norm[0]()   # muls of i-1 (reads accsb, SBUF)
                        pending_norm[0] = None
                    if j == AV_LAG - 1:
                        # alloc late: all readers of the previous acc tiles
                        # (incl. the deferred muls' accsb copies) are emitted.
                        acc[0] = av_ps.tile([128, NI], F32, tag="acc",
                                            name=f"acc_{i}_0")
                        acc[1] = av_ps.tile([128, NI], F32, tag="acc",
                                            name=f"acc_{i}_1")
                    if j == 4:
                        flush_outproj()     # outproj of i-1 (raw ready)
                    if j >= AV_LAG:
                        emit_av(j - AV_LAG)
                for j in range(n_j - AV_LAG, n_j):
                    emit_av(j)

                # normalize part 1: stage each head's 33 acc rows to a base-0
                # SBUF tile on ScalarE (frees the PSUM acc at i-end so the
                # next i's alloc doesn't stall on the deferred muls), then
                # stage the l row to base-0 via a standard DVE copy (custom
                # DVE ops need base-0 SBUF inputs), reciprocal, broadcast.
                # Part 2 (the muls) is deferred into the next i's rounds.
                accsb = []
                bcs = []
                for p in range(2):
                    for hh in range(2):
                        ah = at_pool.tile([HEAD_DIM + 1, NI], F32,
                                          tag="accsb", bufs=8,
                                          name=f"accsb_{i}_{2*p+hh}")
                        nc.scalar.copy(
                            out=ah, in_=acc[p][64 * hh:64 * hh + HEAD_DIM + 1, :])
                        accsb.append(ah)
                for h in range(4):
                    lr = at_pool.tile([1, NI], F32, tag="lr", bufs=8)
                    rc = at_pool.tile([1, NI], F32, tag="rc", bufs=8)
                    bc = at_pool.tile([32, NI], F32, tag="bc", bufs=8)
                    nc.vector.tensor_copy(
                        out=lr, in_=accsb[h][HEAD_DIM:HEAD_DIM + 1, :])
                    nc.vector.reciprocal_approx_fast(out=rc, in_=lr)
                    nc.gpsimd.partition_broadcast(bc, rc)
                    bcs.append(bc)

                def make_muls(i_, accsb_, bcs_, raw_):
                    def muls():
                        for h in range(4):
                            nc.vector.tensor_mul(
                                out=raw_[ts(h, 32), :],
                                in0=accsb_[h][0:HEAD_DIM, :],
                                in1=bcs_[h],
                            )
                    return muls

                pending_norm[0] = make_muls(i, accsb, bcs, raw_sb)
                pending_out.append((i, raw_sb))

            pending_norm[0]()
            flush_outproj()

    nc.compile()
    return nc


_NC_CACHE = None


def _get_nc():
    global _NC_CACHE
    if _NC_CACHE is None:
        _NC_CACHE = build_nc()
    return _NC_CACHE


def _shard_inputs(query, context, Wq, Wk, Wv, Wo):
    query = np.asarray(query, dtype=np.float32)
    context = np.asarray(context, dtype=np.float32)
    Wq = np.asarray(Wq, dtype=np.float32)
    Wk = np.asarray(Wk, dtype=np.float32)
    Wv = np.asarray(Wv, dtype=np.float32)
    Wo = np.asarray(Wo, dtype=np.float32)
    b = query.shape[0]
    in_maps = []
    for core in range(N_CORES):
        bb, p = divmod(core, 2)
        sl = slice(128 * p, 128 * (p + 1))
        in_maps.append({
            "x": np.ascontiguousarray(query[bb].reshape(DIM, N).astype(np.float16)),
            "c": np.ascontiguousarray(context[bb].reshape(DIM, N).astype(np.float16)),
            "wqt": np.ascontiguousarray((Wq[sl, :] * PRESCALE).T.astype(np.float16)),
            "wkt": np.ascontiguousarray(Wk[sl, :].T.astype(np.float16)),
            "wvt": np.ascontiguousarray(Wv[sl, :].T.astype(np.float16)),
            "wot": np.ascontiguousarray(Wo[:, sl].T.astype(np.float16)),
        })
    return in_maps, b


def _run(inputs, trace=False, **kw):
    in_maps, b = _shard_inputs(**inputs)
    nc = _get_nc()
    res = run_bass_kernel_spmd(nc, in_maps, core_ids=list(range(N_CORES)),
                               trace=trace, **kw)
    outs = []
    for bb in range(b):
        y = (res.results[2 * bb]["y"].astype(np.float32)
             + res.results[2 * bb + 1]["y"].astype(np.float32))
        outs.append(y.reshape(DIM, 64, 64))
    return np.stack(outs).astype(np.float32), res


def kernel(**inputs):
    out, _ = _run(inputs)
    return out


# revision 21
# speedup vs baseline: 1.1073x; 1.1073x over previous
"""Trainium2 Bass kernel for nn_Attention (dense transformer spatial attention).

Reference computation (per batch b):
    q = Wq @ x   (1x1 conv over channels), k = Wk @ c, v = Wv @ c
    per head h (8 heads, head_dim 32, n = 64*64 = 4096 tokens):
        S = (q_h^T k_h) * DIM**-0.5 ; P = softmax(S, axis=-1) ; o_h = v_h P^T
    out = Wo @ concat(o_h)

Sharding (8 cores): core c handles batch b = c//2 and heads 4*(c%2) .. +4
(tensor-parallel over heads).  The two cores of a batch produce partial
outputs Y = Wo_slice @ o_slice which the host sums (fp16 partials, fp32 sum).

Per-core dataflow:
  - The exp() of softmax is the bottleneck: 67M elements/core must be read
    from fp32 PSUM by ScalarE/VectorE (the only engines with PSUM access).
    Work is split: ScalarE runs native exp ACTIVATE on heads {0,1};
    VectorE computes heads {2,3} via a one-instruction Schraudolph exp2:
    the scores arrive pre-scaled by 1024*log2(e)*DIM^-0.5 (folded into Wq
    host-side), so  int16(z + B)  reinterpreted as fp16 bits IS exp(s).
    Every SPLIT_K-th round ScalarE also takes heads {2,3} to balance load.
  - Scores are computed transposed S^T[j,i] with 4-way row-tiled matmuls
    (K=32 per head, tile_position=(32h,0)), 4 slots -> 4 distinct PSUM banks.
  - vT (token-major V with a ones column for the softmax denominator) is
    produced directly by a transposed projection: lhsT=context chunk,
    rhs=Wv^T chunk -> out[token, (h,d)].  No PE transposes needed.
  - AV: col-tiled matmuls (M=33: 32 dims + ones row) accumulate over j in
    fp32 PSUM; normalization by 1/l on evacuation (DVE recip + gpsimd
    broadcast + DVE mul).
  - Output projection: fp16 matmul with WoT; fp16 partials DMA'd out.
"""

import os
import sys

import numpy as np

for _p in ("/opt/trn_rl_repo", "/root/.axon_site/_ro/trn_rl_repo"):
    if os.path.isdir(_p) and _p not in sys.path:
        sys.path.insert(0, _p)

import concourse.bass as bass
import concourse.tile as tile
from concourse import bacc, mybir
from concourse.bass import ts
from concourse.bass_utils import run_bass_kernel_spmd

DIM = 512
HEAD = 8
ATTN_DIM = 256
HEAD_DIM = 32
N = 4096  # 64 * 64 tokens
SCALE = DIM ** -0.5

N_CORES = 8
HEADS_PER_CORE = 4
NI = 512   # i-tile (query tokens per score matmul rhs)
NJ = 128   # j-tile (key tokens per score matmul lhsT)
AV_LAG = 2  # rounds between score emission and its AV consumption

# exp2 prescale folded into Wq: scores arrive as z = 1024*log2(e)*s
PRESCALE = 1024.0 * np.log2(np.e) * SCALE
EXP_SCALE = float(np.log(2.0) / 1024.0)   # ScalarE: exp(z * EXP_SCALE) = exp(s)
SCH_B = 15315.0                            # DVE: fp16bits(exp(s)) ~= int16(z + B)
SPLIT_K = 12                               # every k-th round ScalarE takes tB too

F32 = mybir.dt.float32
F16 = mybir.dt.float16
I16 = mybir.dt.int16


def build_nc():
    nc = bacc.Bacc()

    x_d = nc.dram_tensor("x", [DIM, N], F16, kind="ExternalInput").ap()
    c_d = nc.dram_tensor("c", [DIM, N], F16, kind="ExternalInput").ap()
    wqt_d = nc.dram_tensor("wqt", [DIM, 128], F16, kind="ExternalInput").ap()
    wkt_d = nc.dram_tensor("wkt", [DIM, 128], F16, kind="ExternalInput").ap()
    wvt_d = nc.dram_tensor("wvt", [DIM, 128], F16, kind="ExternalInput").ap()
    wot_d = nc.dram_tensor("wot", [128, DIM], F16, kind="ExternalInput").ap()
    y_d = nc.dram_tensor("y", [DIM, N], F16, kind="ExternalOutput").ap()

    from contextlib import ExitStack

    with tile.TileContext(nc) as tc, ExitStack() as stk:
        persist = stk.enter_context(tc.tile_pool(name="persist", bufs=1))

        q_sb = persist.tile([128, N], F16)
        k_sb = persist.tile([128, N], F16)
        # vT: (token-in-chunk, j_chunk, head, 32 dims + ones col)
        vT_sb = persist.tile([128, N // NJ, HEADS_PER_CORE, HEAD_DIM + 1], F16)
        wot_sb = persist.tile([128, DIM], F16)

        nc.sync.dma_start(out=wot_sb, in_=wot_d)
        nc.vector.memset(vT_sb[:, :, :, HEAD_DIM:], 1.0)

        # Preload the exp activation table during the DMA lead-in so the
        # first real exp doesn't pay the ~2.7us ACT_TABLE_LOAD.
        warm_sb = persist.tile([1, 32], F32)
        nc.vector.memset(warm_sb, 0.0)
        nc.scalar.activation(out=warm_sb, in_=warm_sb,
                             func=mybir.ActivationFunctionType.Exp)

        # ---------------- Phase 1: projections ----------------
        cw_pool = stk.enter_context(tc.tile_pool(name="cw", bufs=1))

        w_sb = {}
        for nm, d in (("wkt", wkt_d), ("wqt", wqt_d), ("wvt", wvt_d)):
            w = cw_pool.tile([128, 4, 128], F16, tag=nm)
            nc.sync.dma_start(out=w, in_=d.rearrange("(c p) m -> p c m", p=128))
            w_sb[nm] = w
        # Inputs arrive as [128, 1024] piece tiles spread over four engine
        # DMA queues so projections start within ~2us instead of waiting for
        # full 1MB chunks on one queue.
        n_w = N // (2 * NI)  # 4 pieces per channel-chunk
        c_t = [[None] * n_w for _ in range(4)]
        x_t = [[None] * n_w for _ in range(4)]

        with tc.tile_pool(name="x_in", bufs=1) as x_pool, \
             tc.tile_pool(name="pj_ps", bufs=2, space="PSUM") as pj_ps, \
             tc.tile_pool(name="vt_ps", bufs=4, space="PSUM") as vt_ps:
            for w in range(n_w):
                for cc in range(4):
                    t = cw_pool.tile([128, 2 * NI], F16, tag="c_in", bufs=16)
                    q_eng = nc.sync if cc % 2 == 0 else nc.scalar
                    q_eng.dma_start(out=t, in_=c_d[ts(cc, 128), ts(w, 2 * NI)])
                    c_t[cc][w] = t
                    t = x_pool.tile([128, 2 * NI], F16, tag="x_in", bufs=16)
                    nc.gpsimd.dma_start(out=t, in_=x_d[ts(cc, 128), ts(w, 2 * NI)])
                    x_t[cc][w] = t

            def project_tile(wname, src, dst, t):
                w = w_sb[wname]
                ps = pj_ps.tile([128, NI], F32, tag="pj")
                for cc in range(4):
                    nc.tensor.matmul(
                        ps, lhsT=w[:, cc, :],
                        rhs=src[cc][t // 2][:, ts(t % 2, NI)],
                        start=(cc == 0), stop=(cc == 3),
                    )
                # evacuation split across both PSUM-capable engines
                if t % 2 == 0:
                    nc.scalar.copy(out=dst[:, ts(t, NI)], in_=ps)
                else:
                    nc.vector.tensor_copy(out=dst[:, ts(t, NI)], in_=ps)

            for t in range(N // NI):
                project_tile("wkt", c_t, k_sb, t)
                project_tile("wqt", x_t, q_sb, t)

            # vT via transposed projection: out[token, (h,d)]
            wv = w_sb["wvt"]
            for t in range(N // NJ):
                ps = vt_ps.tile([128, 128], F32, tag="vt")
                for cc in range(4):
                    nc.tensor.matmul(
                        ps,
                        lhsT=c_t[cc][t // 8][:, ts(t % 8, NJ)],
                        rhs=wv[:, cc, :],
                        start=(cc == 0), stop=(cc == 3),
                    )
                src = ps.rearrange("p (h d) -> p h d", h=HEADS_PER_CORE)
                if t % 2 == 0:
                    nc.scalar.copy(out=vT_sb[:, t, :, 0:HEAD_DIM], in_=src)
                else:
                    nc.vector.tensor_copy(out=vT_sb[:, t, :, 0:HEAD_DIM], in_=src)

        # ---------------- Phase 2: attention ----------------
        with tc.tile_pool(name="sc_ps", bufs=3, space="PSUM") as sc_ps, \
             tc.tile_pool(name="av_ps", bufs=2, space="PSUM") as av_ps, \
             tc.tile_pool(name="attn", bufs=1) as at_pool:

            n_i = N // NI   # 8
            n_j = N // NJ   # 32

            pending_out = []
            round_no = [0]

            def flush_outproj():
                while pending_out:
                    oi, rsb = pending_out.pop(0)
                    for half in range(2):
                        ps = sc_ps.tile([128, 2 * NI], F32, tag="sc",
                                        name=f"ofps_{oi}_{half}")
                        for cc in range(2):
                            c4 = 2 * half + cc
                            nc.tensor.matmul(
                                ps[:, ts(cc, NI)],
                                lhsT=wot_sb[:, ts(c4, 128)], rhs=rsb,
                                start=True, stop=True,
                            )
                        ot = at_pool.tile([128, 2 * NI], F16, tag="ot", bufs=4,
                                          name=f"ot_{oi}_{half}")
                        if half == 0:
                            nc.scalar.copy(out=ot, in_=ps)
                        else:
                            nc.vector.tensor_copy(out=ot, in_=ps)
                        for cc in range(2):
                            c4 = 2 * half + cc
                            nc.sync.dma_start(
                                out=y_d[ts(c4, 128), ts(oi, NI)],
                                in_=ot[:, ts(cc, NI)])

            pending_norm = [None]

            for i in range(n_i):
                raw_sb = at_pool.tile([128, NI], F16, tag="raw", bufs=2)
                es_ref = [None] * n_j  # j -> (esA, esB)
                acc = [None, None]

                def emit_round(j):
                    # alternate (tA, tB) allocation order so the pool
                    # rotation's 3-back dependency lands on the same engine
                    # half the time instead of always cross-coupling S and D.
                    tA = sc_ps.tile([128, 2 * NI], F32, tag="sc",
                                    name=f"scA_{i}_{j}")
                    tB = sc_ps.tile([128, 2 * NI], F32, tag="sc",
                                    name=f"scB_{i}_{j}")
                    if j % 2 == 1:
                        tA, tB = tB, tA
                    # ScalarE is the critical engine: its tile's matmuls
                    # (heads 0,1) always go first so the ACT starts earliest
                    for h in range(4):
                        dst = (tA if h < 2 else tB)[:, ts(h % 2, NI)]
                        base = 32 * h
                        nc.tensor.matmul(
                            dst,
                            lhsT=k_sb[base:base + 32, ts(j, NJ)],
                            rhs=q_sb[base:base + 32, ts(i, NI)],
                            start=True, stop=True,
                            tile_position=(base, 0),
                        )
                    esA = at_pool.tile([128, 2 * NI], F16, tag="es",
                                       bufs=12, name=f"esA_{i}_{j}")
                    esB = at_pool.tile([128, 2 * NI], F16, tag="es",
                                       bufs=12, name=f"esB_{i}_{j}")
                    nc.scalar.activation(
                        out=esA, in_=tA,
                        func=mybir.ActivationFunctionType.Exp,
                        scale=EXP_SCALE,
                    )
                    r = round_no[0]
                    round_no[0] += 1
                    if r % SPLIT_K == SPLIT_K - 1:
                        nc.scalar.activation(
                            out=esB, in_=tB,
                            func=mybir.ActivationFunctionType.Exp,
                            scale=EXP_SCALE,
                        )
                    else:
                        nc.vector.tensor_scalar(
                            out=esB.bitcast(I16), in0=tB,
                            scalar1=SCH_B, scalar2=None,
                            op0=mybir.AluOpType.add,
                        )
                    es_ref[j] = (esA, esB)

                def emit_av(j):
                    esA, esB = es_ref[j]
                    for p in range(2):
                        es = esA if p == 0 else esB
                        for hh in range(2):
                            nc.tensor.matmul(
                                acc[p][64 * hh:64 * hh + HEAD_DIM + 1, :],
                                lhsT=vT_sb[:, j, 2 * p + hh, :],
                                rhs=es[:, ts(hh, NI)],
                                start=(j == 0), stop=(j == n_j - 1),
                                tile_position=(0, 64 * hh),
                                skip_group_check=True,
                            )

                for j in range(n_j):
                    emit_round(j)
                    if j == 1 and pending_norm[0] is not None:
                        pending_norm[0]()   # muls of i-1 (reads accsb, SBUF)
                        pending_norm[0] = None
                    if j == AV_LAG - 1:
                        # alloc late: all readers of the previous acc tiles
                        # (incl. the deferred muls' accsb copies) are emitted.
                        acc[0] = av_ps.tile([128, NI], F32, tag="acc",
                                            name=f"acc_{i}_0")
                        acc[1] = av_ps.tile([128, NI], F32, tag="acc",
                                            name=f"acc_{i}_1")
                    if j == 4:
                        flush_outproj()     # outproj of i-1 (raw ready)
                    if j >= AV_LAG:
                        emit_av(j - AV_LAG)
                for j in range(n_j - AV_LAG, n_j):
                    emit_av(j)

                # normalize part 1: stage each head's 33 acc rows to a base-0
                # SBUF tile on ScalarE (frees the PSUM acc at i-end so the
                # next i's alloc doesn't stall on the deferred muls), then
                # stage the l row to base-0 via a standard DVE copy (custom
                # DVE ops need base-0 SBUF inputs), reciprocal, broadcast.
                # Part 2 (the muls) is deferred into the next i's rounds.
                accsb = []
                bcs = []
                for p in range(2):
                    for hh in range(2):
                        ah = at_pool.tile([HEAD_DIM + 1, NI], F32,
                                          tag="accsb", bufs=8,
                                          name=f"accsb_{i}_{2*p+hh}")
                        nc.scalar.copy(
                            out=ah, in_=acc[p][64 * hh:64 * hh + HEAD_DIM + 1, :])
                        accsb.append(ah)
                for h in range(4):
                    lr = at_pool.tile([1, NI], F32, tag="lr", bufs=8)
                    rc = at_pool.tile([1, NI], F32, tag="rc", bufs=8)
                    bc = at_pool.tile([32, NI], F32, tag="bc", bufs=8)
                    nc.vector.tensor_copy(
                        out=lr, in_=accsb[h][HEAD_DIM:HEAD_DIM + 1, :])
                    nc.vector.reciprocal_approx_fast(out=rc, in_=lr)
                    nc.gpsimd.partition_broadcast(bc, rc)
                    bcs.append(bc)

                def make_muls(i_, accsb_, bcs_, raw_):
                    def muls():
                        for h in range(4):
                            nc.vector.tensor_mul(
                                out=raw_[ts(h, 32), :],
                                in0=accsb_[h][0:HEAD_DIM, :],
                                in1=bcs_[h],
                            )
                    return muls

                pending_norm[0] = make_muls(i, accsb, bcs, raw_sb)
                pending_out.append((i, raw_sb))

            pending_norm[0]()
            flush_outproj()

    nc.compile()
    return nc


_NC_CACHE = None


def _get_nc():
    global _NC_CACHE
    if _NC_CACHE is None:
        _NC_CACHE = build_nc()
    return _NC_CACHE


def _shard_inputs(query, context, Wq, Wk, Wv, Wo):
    query = np.asarray(query, dtype=np.float32)
    context = np.asarray(context, dtype=np.float32)
    Wq = np.asarray(Wq, dtype=np.float32)
    Wk = np.asarray(Wk, dtype=np.float32)
    Wv = np.asarray(Wv, dtype=np.float32)
    Wo = np.asarray(Wo, dtype=np.float32)
    b = query.shape[0]
    in_maps = []
    for core in range(N_CORES):
        bb, p = divmod(core, 2)
        sl = slice(128 * p, 128 * (p + 1))
        in_maps.append({
            "x": np.ascontiguousarray(query[bb].reshape(DIM, N).astype(np.float16)),
            "c": np.ascontiguousarray(context[bb].reshape(DIM, N).astype(np.float16)),
            "wqt": np.ascontiguousarray((Wq[sl, :] * PRESCALE).T.astype(np.float16)),
            "wkt": np.ascontiguousarray(Wk[sl, :].T.astype(np.float16)),
            "wvt": np.ascontiguousarray(Wv[sl, :].T.astype(np.float16)),
            "wot": np.ascontiguousarray(Wo[:, sl].T.astype(np.float16)),
        })
    return in_maps, b


def _run(inputs, trace=False, **kw):
    in_maps, b = _shard_inputs(**inputs)
    nc = _get_nc()
    res = run_bass_kernel_spmd(nc, in_maps, core_ids=list(range(N_CORES)),
                               trace=trace, **kw)
    outs = []
    for bb in range(b):
        y = (res.results[2 * bb]["y"].astype(np.float32)
             + res.results[2 * bb + 1]["y"].astype(np.float32))
        outs.append(y.reshape(DIM, 64, 64))
    return np.stack(outs).astype(np.float32), res


def kernel(**inputs):
    out, _ = _run(inputs)
    return out


# revision 26
# speedup vs baseline: 1.1819x; 1.0674x over previous
"""Trainium2 Bass kernel for nn_Attention (dense transformer spatial attention).

Reference computation (per batch b):
    q = Wq @ x   (1x1 conv over channels), k = Wk @ c, v = Wv @ c
    per head h (8 heads, head_dim 32, n = 64*64 = 4096 tokens):
        S = (q_h^T k_h) * DIM**-0.5 ; P = softmax(S, axis=-1) ; o_h = v_h P^T
    out = Wo @ concat(o_h)

Sharding (8 cores): core c handles batch b = c//2 and heads 4*(c%2) .. +4
(tensor-parallel over heads).  The two cores of a batch produce partial
outputs Y = Wo_slice @ o_slice which the host sums (fp16 partials, fp32 sum).

Per-core dataflow:
  - The exp() of softmax is the bottleneck: 67M elements/core must be read
    from fp32 PSUM by ScalarE/VectorE (the only engines with PSUM access).
    Work is split: ScalarE runs native exp ACTIVATE on heads {0,1};
    VectorE computes heads {2,3} via a one-instruction Schraudolph exp2:
    the scores arrive pre-scaled by 1024*log2(e)*DIM^-0.5 (folded into Wq
    host-side), so  int16(z + B)  reinterpreted as fp16 bits IS exp(s).
    Every SPLIT_K-th round ScalarE also takes heads {2,3} to balance load.
  - Scores are computed transposed S^T[j,i] with 4-way row-tiled matmuls
    (K=32 per head, tile_position=(32h,0)), 4 slots -> 4 distinct PSUM banks.
  - vT (token-major V with a ones column for the softmax denominator) is
    produced directly by a transposed projection: lhsT=context chunk,
    rhs=Wv^T chunk -> out[token, (h,d)].  No PE transposes needed.
  - AV: col-tiled matmuls (M=33: 32 dims + ones row) accumulate over j in
    fp32 PSUM; normalization by 1/l on evacuation (DVE recip + gpsimd
    broadcast + DVE mul).
  - Output projection: fp16 matmul with WoT; fp16 partials DMA'd out.
"""

import os
import sys

import numpy as np

for _p in ("/opt/trn_rl_repo", "/root/.axon_site/_ro/trn_rl_repo"):
    if os.path.isdir(_p) and _p not in sys.path:
        sys.path.insert(0, _p)

import concourse.bass as bass
import concourse.tile as tile
from concourse import bacc, mybir
from concourse.bass import ts
from concourse.bass_utils import run_bass_kernel_spmd

DIM = 512
HEAD = 8
ATTN_DIM = 256
HEAD_DIM = 32
N = 4096  # 64 * 64 tokens
SCALE = DIM ** -0.5

N_CORES = 8
HEADS_PER_CORE = 4
NI = 512   # i-tile (query tokens per score matmul rhs)
NJ = 128   # j-tile (key tokens per score matmul lhsT)
AV_LAG = 2  # rounds between score emission and its AV consumption

# exp2 prescale folded into Wq: scores arrive as z = 1024*log2(e)*s
PRESCALE = 1024.0 * np.log2(np.e) * SCALE
EXP_SCALE = float(np.log(2.0) / 1024.0)   # ScalarE: exp(z * EXP_SCALE) = exp(s)
SCH_B = 15315.0                            # DVE: fp16bits(exp(s)) ~= int16(z + B)
SPLIT_K = 12                               # every k-th round ScalarE takes tB too

F32 = mybir.dt.float32
F16 = mybir.dt.float16
I16 = mybir.dt.int16


def build_nc():
    nc = bacc.Bacc()

    x_d = nc.dram_tensor("x", [DIM, N], F16, kind="ExternalInput").ap()
    c_d = nc.dram_tensor("c", [DIM, N], F16, kind="ExternalInput").ap()
    wqt_d = nc.dram_tensor("wqt", [DIM, 128], F16, kind="ExternalInput").ap()
    wkt_d = nc.dram_tensor("wkt", [DIM, 128], F16, kind="ExternalInput").ap()
    wvt_d = nc.dram_tensor("wvt", [DIM, 128], F16, kind="ExternalInput").ap()
    wot_d = nc.dram_tensor("wot", [128, DIM], F16, kind="ExternalInput").ap()
    y_d = nc.dram_tensor("y", [DIM, N], F16, kind="ExternalOutput").ap()

    from contextlib import ExitStack

    with tile.TileContext(nc) as tc, ExitStack() as stk:
        persist = stk.enter_context(tc.tile_pool(name="persist", bufs=1))

        q_sb = persist.tile([128, N], F16)
        k_sb = persist.tile([128, N], F16)
        # vT: (token-in-chunk, j_chunk, head, 32 dims + ones col)
        vT_sb = persist.tile([128, N // NJ, HEADS_PER_CORE, HEAD_DIM + 1], F16)
        wot_sb = persist.tile([128, DIM], F16)

        nc.vector.memset(vT_sb[:, :, :, HEAD_DIM:], 1.0)

        # Preload the exp activation table during the DMA lead-in so the
        # first real exp doesn't pay the ~2.7us ACT_TABLE_LOAD.
        warm_sb = persist.tile([1, 32], F32)
        nc.vector.memset(warm_sb, 0.0)
        nc.scalar.activation(out=warm_sb, in_=warm_sb,
                             func=mybir.ActivationFunctionType.Exp)

        # ---------------- Phase 1: projections ----------------
        cw_pool = stk.enter_context(tc.tile_pool(name="cw", bufs=1))

        # weights as per-chunk contiguous DMAs (the single rearranged gather
        # generates tiny strided descriptors and takes ~5x longer), ordered
        # by first use: wkt (sync) and wqt (scalar) gate the projections;
        # wvt/wot follow.
        w_sb = {}
        for nm, d, q_eng in (("wkt", wkt_d, nc.sync), ("wqt", wqt_d, nc.scalar)):
            w = cw_pool.tile([128, 4, 128], F16, tag=nm)
            for cc in range(4):
                q_eng.dma_start(out=w[:, cc, :], in_=d[ts(cc, 128), :])
            w_sb[nm] = w
        # Inputs arrive as [128, 1024] piece tiles spread over four engine
        # DMA queues so projections start within ~2us instead of waiting for
        # full 1MB chunks on one queue.
        n_w = N // (2 * NI)  # 4 pieces per channel-chunk
        c_t = [[None] * n_w for _ in range(4)]
        x_t = [[None] * n_w for _ in range(4)]

        with tc.tile_pool(name="x_in", bufs=1) as x_pool, \
             tc.tile_pool(name="pj_ps", bufs=2, space="PSUM") as pj_ps, \
             tc.tile_pool(name="vt_ps", bufs=4, space="PSUM") as vt_ps:
            for w in range(n_w):
                for cc in range(4):
                    t = cw_pool.tile([128, 2 * NI], F16, tag="c_in", bufs=16)
                    q_eng = nc.sync if cc % 2 == 0 else nc.scalar
                    q_eng.dma_start(out=t, in_=c_d[ts(cc, 128), ts(w, 2 * NI)])
                    c_t[cc][w] = t
                    t = x_pool.tile([128, 2 * NI], F16, tag="x_in", bufs=16)
                    nc.gpsimd.dma_start(out=t, in_=x_d[ts(cc, 128), ts(w, 2 * NI)])
                    x_t[cc][w] = t

            # late-use weights after the projection-gating input pieces
            wv = cw_pool.tile([128, 4, 128], F16, tag="wvt")
            for cc in range(4):
                nc.scalar.dma_start(out=wv[:, cc, :], in_=wvt_d[ts(cc, 128), :])
            w_sb["wvt"] = wv
            nc.gpsimd.dma_start(out=wot_sb, in_=wot_d)

            def project_tile(wname, src, dst, t):
                w = w_sb[wname]
                ps = pj_ps.tile([128, NI], F32, tag="pj")
                for cc in range(4):
                    nc.tensor.matmul(
                        ps, lhsT=w[:, cc, :],
                        rhs=src[cc][t // 2][:, ts(t % 2, NI)],
                        start=(cc == 0), stop=(cc == 3),
                    )
                # evacuation split across both PSUM-capable engines
                if t % 2 == 0:
                    nc.scalar.copy(out=dst[:, ts(t, NI)], in_=ps)
                else:
                    nc.vector.tensor_copy(out=dst[:, ts(t, NI)], in_=ps)

            for t in range(N // NI):
                project_tile("wkt", c_t, k_sb, t)
                project_tile("wqt", x_t, q_sb, t)

            # vT via transposed projection: out[token, (h,d)]
            wv = w_sb["wvt"]
            for t in range(N // NJ):
                ps = vt_ps.tile([128, 128], F32, tag="vt")
                for cc in range(4):
                    nc.tensor.matmul(
                        ps,
                        lhsT=c_t[cc][t // 8][:, ts(t % 8, NJ)],
                        rhs=wv[:, cc, :],
                        start=(cc == 0), stop=(cc == 3),
                    )
                src = ps.rearrange("p (h d) -> p h d", h=HEADS_PER_CORE)
                if t % 2 == 0:
                    nc.scalar.copy(out=vT_sb[:, t, :, 0:HEAD_DIM], in_=src)
                else:
                    nc.vector.tensor_copy(out=vT_sb[:, t, :, 0:HEAD_DIM], in_=src)

        # ---------------- Phase 2: attention ----------------
        with tc.tile_pool(name="sc_ps", bufs=3, space="PSUM") as sc_ps, \
             tc.tile_pool(name="av_ps", bufs=2, space="PSUM") as av_ps, \
             tc.tile_pool(name="attn", bufs=1) as at_pool:

            n_i = N // NI   # 8
            n_j = N // NJ   # 32

            pending_out = []
            round_no = [0]

            def flush_outproj():
                while pending_out:
                    oi, rsb = pending_out.pop(0)
                    for half in range(2):
                        ps = sc_ps.tile([128, 2 * NI], F32, tag="sc",
                                        name=f"ofps_{oi}_{half}")
                        for cc in range(2):
                            c4 = 2 * half + cc
                            nc.tensor.matmul(
                                ps[:, ts(cc, NI)],
                                lhsT=wot_sb[:, ts(c4, 128)], rhs=rsb,
                                start=True, stop=True,
                            )
                        ot = at_pool.tile([128, 2 * NI], F16, tag="ot", bufs=4,
                                          name=f"ot_{oi}_{half}")
                        if half == 0:
                            nc.scalar.copy(out=ot, in_=ps)
                        else:
                            nc.vector.tensor_copy(out=ot, in_=ps)
                        for cc in range(2):
                            c4 = 2 * half + cc
                            nc.sync.dma_start(
                                out=y_d[ts(c4, 128), ts(oi, NI)],
                                in_=ot[:, ts(cc, NI)])

            pending_norm = [None]

            for i in range(n_i):
                raw_sb = at_pool.tile([128, NI], F16, tag="raw", bufs=2)
                es_ref = [None] * n_j  # j -> (esA, esB)
                acc = [None, None]

                def emit_round(j):
                    # alternate (tA, tB) allocation order so the pool
                    # rotation's 3-back dependency lands on the same engine
                    # half the time instead of always cross-coupling S and D.
                    tA = sc_ps.tile([128, 2 * NI], F32, tag="sc",
                                    name=f"scA_{i}_{j}")
                    tB = sc_ps.tile([128, 2 * NI], F32, tag="sc",
                                    name=f"scB_{i}_{j}")
                    if j % 2 == 1:
                        tA, tB = tB, tA
                    # ScalarE is the critical engine: its tile's matmuls
                    # (heads 0,1) always go first so the ACT starts earliest
                    for h in range(4):
                        dst = (tA if h < 2 else tB)[:, ts(h % 2, NI)]
                        base = 32 * h
                        nc.tensor.matmul(
                            dst,
                            lhsT=k_sb[base:base + 32, ts(j, NJ)],
                            rhs=q_sb[base:base + 32, ts(i, NI)],
                            start=True, stop=True,
                            tile_position=(base, 0),
                        )
                    esA = at_pool.tile([128, 2 * NI], F16, tag="es",
                                       bufs=12, name=f"esA_{i}_{j}")
                    esB = at_pool.tile([128, 2 * NI], F16, tag="es",
                                       bufs=12, name=f"esB_{i}_{j}")
                    nc.scalar.activation(
                        out=esA, in_=tA,
                        func=mybir.ActivationFunctionType.Exp,
                        scale=EXP_SCALE,
                    )
                    r = round_no[0]
                    round_no[0] += 1
                    if r % SPLIT_K == SPLIT_K - 1:
                        nc.scalar.activation(
                            out=esB, in_=tB,
                            func=mybir.ActivationFunctionType.Exp,
                            scale=EXP_SCALE,
                        )
                    else:
                        nc.vector.tensor_scalar(
                            out=esB.bitcast(I16), in0=tB,
                            scalar1=SCH_B, scalar2=None,
                            op0=mybir.AluOpType.add,
                        )
                    es_ref[j] = (esA, esB)

                def emit_av(j):
                    esA, esB = es_ref[j]
                    for p in range(2):
                        es = esA if p == 0 else esB
                        for hh in range(2):
                            nc.tensor.matmul(
                                acc[p][64 * hh:64 * hh + HEAD_DIM + 1, :],
                                lhsT=vT_sb[:, j, 2 * p + hh, :],
                                rhs=es[:, ts(hh, NI)],
                                start=(j == 0), stop=(j == n_j - 1),
                                tile_position=(0, 64 * hh),
                                skip_group_check=True,
                            )

                for j in range(n_j):
                    emit_round(j)
                    if j == 1 and pending_norm[0] is not None:
                        pending_norm[0]()   # muls of i-1 (reads accsb, SBUF)
                        pending_norm[0] = None
                    if j == AV_LAG - 1:
                        # alloc late: all readers of the previous acc tiles
                        # (incl. the deferred muls' accsb copies) are emitted.
                        acc[0] = av_ps.tile([128, NI], F32, tag="acc",
                                            name=f"acc_{i}_0")
                        acc[1] = av_ps.tile([128, NI], F32, tag="acc",
                                            name=f"acc_{i}_1")
                    if j == 4:
                        flush_outproj()     # outproj of i-1 (raw ready)
                    if j >= AV_LAG:
                        emit_av(j - AV_LAG)
                for j in range(n_j - AV_LAG, n_j):
                    emit_av(j)

                # normalize part 1: stage the l rows to base-0 SBUF tiles on
                # ScalarE (custom-DVE ops corrupt rare values on PSUM or
                # base-shifted inputs — keep their reads base-0 SBUF), then
                # reciprocals + broadcasts.  Part 2 (the muls, reading the
                # PSUM acc directly) is deferred into the next i's rounds.
                bcs = []
                for p in range(2):
                    for hh in range(2):
                        lrow = acc[p][64 * hh + HEAD_DIM:64 * hh + HEAD_DIM + 1, :]
                        lr = at_pool.tile([1, NI], F32, tag="lr", bufs=8)
                        rc = at_pool.tile([1, NI], F32, tag="rc", bufs=8)
                        bc = at_pool.tile([32, NI], F32, tag="bc", bufs=8)
                        nc.scalar.copy(out=lr, in_=lrow)
                        nc.vector.reciprocal_approx_fast(out=rc, in_=lr)
                        nc.gpsimd.partition_broadcast(bc, rc)
                        bcs.append(bc)

                def make_muls(i_, acc_, bcs_, raw_):
                    def muls():
                        for p in range(2):
                            for hh in range(2):
                                h = 2 * p + hh
                                nc.vector.tensor_mul(
                                    out=raw_[ts(h, 32), :],
                                    in0=acc_[p][64 * hh:64 * hh + 32, :],
                                    in1=bcs_[h],
                                )
                    return muls

                pending_norm[0] = make_muls(i, acc, bcs, raw_sb)
                pending_out.append((i, raw_sb))

            pending_norm[0]()
            flush_outproj()

    nc.compile()
    return nc


_NC_CACHE = None


def _get_nc():
    global _NC_CACHE
    if _NC_CACHE is None:
        _NC_CACHE = build_nc()
    return _NC_CACHE


def _shard_inputs(query, context, Wq, Wk, Wv, Wo):
    query = np.asarray(query, dtype=np.float32)
    context = np.asarray(context, dtype=np.float32)
    Wq = np.asarray(Wq, dtype=np.float32)
    Wk = np.asarray(Wk, dtype=np.float32)
    Wv = np.asarray(Wv, dtype=np.float32)
    Wo = np.asarray(Wo, dtype=np.float32)
    b = query.shape[0]
    in_maps = []
    for core in range(N_CORES):
        bb, p = divmod(core, 2)
        sl = slice(128 * p, 128 * (p + 1))
        in_maps.append({
            "x": np.ascontiguousarray(query[bb].reshape(DIM, N).astype(np.float16)),
            "c": np.ascontiguousarray(context[bb].reshape(DIM, N).astype(np.float16)),
            "wqt": np.ascontiguousarray((Wq[sl, :] * PRESCALE).T.astype(np.float16)),
            "wkt": np.ascontiguousarray(Wk[sl, :].T.astype(np.float16)),
            "wvt": np.ascontiguousarray(Wv[sl, :].T.astype(np.float16)),
            "wot": np.ascontiguousarray(Wo[:, sl].T.astype(np.float16)),
        })
    return in_maps, b


def _run(inputs, trace=False, **kw):
    in_maps, b = _shard_inputs(**inputs)
    nc = _get_nc()
    res = run_bass_kernel_spmd(nc, in_maps, core_ids=list(range(N_CORES)),
                               trace=trace, **kw)
    outs = []
    for bb in range(b):
        y = (res.results[2 * bb]["y"].astype(np.float32)
             + res.results[2 * bb + 1]["y"].astype(np.float32))
        outs.append(y.reshape(DIM, 64, 64))
    return np.stack(outs).astype(np.float32), res


def kernel(**inputs):
    out, _ = _run(inputs)
    return out
